# revision 1
# baseline (speedup 1.0000x reference)
"""ObjectDecoder kernel for Trainium2 (8 NeuronCores, data-parallel over batch).

Computes out[b, o, a, p, k] = sum_d x[b, o, d] * W[o, a, p, d, k] + bias[o, a, p, k]
  x: [16384, 16, 256] f32, W: [16, 4, 2, 256, 8] f32, b: [16, 4, 2, 8] f32
  out: [16384, 16, 4, 2, 8] f32

Per-core plan (batch shard of 2048 rows):
  - x shard is pre-transposed on host to xt[obj, d, batch] so the contraction
    dim (d) lands on SBUF partitions and every DMA is a large contiguous block.
  - W is pre-arranged to wt[d_lo(128), k_chunk(2), obj(16), apk(64)]; bias to
    bt[(pair_half*64+apk)(128), pair(8)].
  - For each pair of objects: 4 matmuls [K=128, M=64, N=512] accumulate into a
    [128, 512] PSUM bank (objects 2i / 2i+1 stacked on partitions); the scalar
    engine evacuates PSUM with a fused per-partition bias add; result stores to
    out_t[obj, apk, batch] in DRAM, un-transposed on host at the end.
"""

import os
from contextlib import ExitStack

os.environ.setdefault("JAX_PLATFORMS", "axon")

import numpy as np

import concourse.bass as bass
import concourse.mybir as mybir
import concourse.tile as tile
from concourse import bacc
from concourse.bass_utils import run_bass_kernel_spmd

B, N_OBJ, DIM_IN, APK = 16384, 16, 256, 64
N_CORES = 8
BS = B // N_CORES          # 2048 batch rows per core
NT = 512                   # moving-operand tile (fp32 max, one PSUM bank)
NB = BS // NT              # 4 batch chunks per core
F32 = mybir.dt.float32
F32R = mybir.dt.float32r

_CACHE: dict = {}


def _build_nc(variant=None):
    if variant is None:
        variant = os.environ.get("KVARIANT", "v5")
    nc = bacc.Bacc("TRN2", target_bir_lowering=False, debug=False)

    # xt[o, p, k, b]: d = k*128 + p — 16KiB contiguous per partition line
    xt = nc.declare_dram_parameter("xt", [N_OBJ, 128, 2, BS], F32, isOutput=False)
    wt = nc.declare_dram_parameter("wt", [128, 2, N_OBJ, APK], F32, isOutput=False)
    bt = nc.declare_dram_parameter("bt", [128, N_OBJ // 2], F32, isOutput=False)
    out = nc.declare_dram_parameter("out", [N_OBJ, APK, BS], F32, isOutput=True)

    with tile.TileContext(nc) as tc, ExitStack() as ctx:
        wpool = ctx.enter_context(tc.tile_pool(name="w", bufs=1))
        n_fine = 2 if variant == "v4" else 1
        xpool = ctx.enter_context(
            tc.tile_pool(name="x", bufs=5 if variant == "v4" else 6)
        )
        fpool = ctx.enter_context(
            tc.tile_pool(name="xf", bufs=2 * n_fine)
        )
        psum = ctx.enter_context(
            tc.tile_pool(name="ps", bufs=8, space=bass.MemorySpace.PSUM)
        )
        opool = ctx.enter_context(tc.tile_pool(name="o", bufs=3))

        # W/bias via SWDGE (gpsimd) — off the busy sync queue, may start
        # during SP's register-init window
        w_sb = wpool.tile([128, 2, N_OBJ, APK], F32)
        nc.gpsimd.dma_start(w_sb[:], wt[:])
        b_sb = wpool.tile([128, N_OBJ // 2], F32)
        nc.gpsimd.dma_start(b_sb[:], bt[:])

        n_pairs = N_OBJ // 2
        for op in range(n_pairs):  # object pairs
            # Last pairs: n-granular loads/stores to shrink the pipeline-drain
            # tail (nothing left to overlap the final compute+stores with).
            fine = op >= n_pairs - n_fine
            xts = {}
            for o2 in range(2):
                pool = fpool if fine else xpool
                t = pool.tile([128, 2, BS], F32)
                if fine and variant == "v4":
                    # per-chunk loads so the tail compute+stores start early
                    for k in range(2):
                        for n in range(NB):
                            nc.sync.dma_start(
                                t[:, k, n * NT : (n + 1) * NT],
                                xt[2 * op + o2, :, k, n * NT : (n + 1) * NT],
                            )
                elif fine:
                    # batch-half loads (4KiB lines)
                    for h in range(2):
                        hs = h * (BS // 2)
                        nc.sync.dma_start(
                            t[:, :, hs : hs + BS // 2],
                            xt[2 * op + o2, :, :, hs : hs + BS // 2],
                        )
                else:
                    nc.sync.dma_start(t[:], xt[2 * op + o2])
                for k in range(2):
                    xts[o2, k] = t[:, k, :]
            ot = opool.tile([128, BS], F32)
            for n in range(NB):
                ps = psum.tile([128, NT], F32)
                for o2 in range(2):
                    for k in range(2):
                        nc.tensor.matmul(
                            ps[o2 * 64 : (o2 + 1) * 64, :],
                            w_sb[:, k, 2 * op + o2, :],
                            xts[o2, k][:, n * NT : (n + 1) * NT],
                            start=(k == 0),
                            stop=(k == 1),
                        )
                nc.scalar.activation(
                    ot[:, n * NT : (n + 1) * NT],
                    ps[:],
                    mybir.ActivationFunctionType.Identity,
                    bias=b_sb[:, op : op + 1],
                )
                if fine:
                    nc.scalar.dma_start(
                        out[2 * op : 2 * op + 2, :, n * NT : (n + 1) * NT],
                        ot[:, n * NT : (n + 1) * NT],
                    )
                elif op == n_pairs - n_fine - 1 and n % 2 == 1:
                    # second-to-last pair: store per batch-half (4KiB lines)
                    # so its store doesn't wait on the whole pair's compute
                    hs = (n - 1) * NT
                    nc.scalar.dma_start(
                        out[2 * op : 2 * op + 2, :, hs : hs + 2 * NT],
                        ot[:, hs : hs + 2 * NT],
                    )
            if not fine and op != n_pairs - n_fine - 1:
                nc.scalar.dma_start(out[2 * op : 2 * op + 2, :, :], ot[:])

    nc.compile()
    return nc


def _get_nc():
    if "nc" not in _CACHE:
        _CACHE["nc"] = _build_nc()
    return _CACHE["nc"]


def _prep_inputs(x, W, b):
    x = np.ascontiguousarray(x, dtype=np.float32)
    # wt[d_lo, k_chunk, o, apk]: W[o,a,p,d,k] -> [d,o,apk] -> [2,128,o,apk] -> [128,2,o,apk]
    wt = np.ascontiguousarray(
        np.asarray(W, dtype=np.float32)
        .transpose(3, 0, 1, 2, 4)
        .reshape(2, 128, N_OBJ, APK)
        .transpose(1, 0, 2, 3)
    )
    # bt[o2*64+apk, pair]
    bt = np.ascontiguousarray(
        np.asarray(b, dtype=np.float32)
        .reshape(N_OBJ // 2, 2, APK)
        .transpose(1, 2, 0)
        .reshape(128, N_OBJ // 2)
    )
    in_maps = []
    for c in range(N_CORES):
        xs = x[c * BS : (c + 1) * BS]  # [BS, 16, 256]
        # xt[o, p, k, b] with d = k*128 + p (16KiB contiguous per (o, p))
        xt = np.ascontiguousarray(
            xs.transpose(1, 2, 0).reshape(N_OBJ, 2, 128, BS).transpose(0, 2, 1, 3)
        )
        in_maps.append({"xt": xt, "wt": wt, "bt": bt})
    return in_maps


def kernel(x, W, b, _trace=False, **run_kwargs):
    nc = _get_nc()
    in_maps = _prep_inputs(x, W, b)
    res = run_bass_kernel_spmd(
        nc, in_maps, core_ids=list(range(N_CORES)), trace=_trace, **run_kwargs
    )
    _CACHE["last_results"] = res
    out = np.empty((B, N_OBJ, APK), dtype=np.float32)
    for c in range(N_CORES):
        # out_t[o, apk, batch] -> [batch, o, apk]
        out[c * BS : (c + 1) * BS] = res.results[c]["out"].transpose(2, 0, 1)
    return out.reshape(B, N_OBJ, 4, 2, 8)



# revision 8
# speedup vs baseline: 2.0100x; 2.0100x over previous
"""ObjectDecoder kernel for Trainium2 (8 NeuronCores, data-parallel over batch).

Computes out[b, o, a, p, k] = sum_d x[b, o, d] * W[o, a, p, d, k] + bias[o, a, p, k]
  x: [16384, 16, 256] f32, W: [16, 4, 2, 256, 8] f32, b: [16, 4, 2, 8] f32
  out: [16384, 16, 4, 2, 8] f32

DMA-bound problem: per core the batch shard is 2048 rows -> 33.5 MB of x in
fp32. The 2e-2 rel-err budget is ~5000x above fp32 accuracy, so x/W/out all
move as bf16 (measured end-to-end rel err ~3.5e-3), halving HBM bytes.

Per-core plan (batch shard of 2048 rows):
  - x pre-packed on host to xt[pair, p(128), 8192] bf16 where d = k*128 + p and
    the free dim is [o2, k, b] for mid pairs (16 KiB contiguous per partition
    line) or [ch, o2, k, b%1024] chunk-major for the first/last pair (8 KiB
    granules so head/tail compute overlaps the edge loads).
  - All 10 x dma_starts are issued upfront on the sync queue: the 16 DMA
    engines then stream the full 16.8 MB back-to-back with no dependency
    stalls; compute chases the loads.
  - Per pair: 16 matmuls [K=128, M=64, N=512] (bf16, fp32 PSUM accumulate),
    k-outer order so the stationary operand switches only 4x per pair.
    Dummy matmuls on scratch SBUF at t=0 ramp the PE clock out of its
    p-state before the first real matmul.
  - Scalar engine evacuates PSUM with fused per-partition bias add to bf16;
    stores go out on the vector queue, two pairs per dma_start (8 KiB lines).
"""

import os
from contextlib import ExitStack

os.environ.setdefault("JAX_PLATFORMS", "axon")

import ml_dtypes
import numpy as np

import concourse.bass as bass
import concourse.mybir as mybir
import concourse.tile as tile
from concourse import bacc
from concourse.bass_utils import run_bass_kernel_spmd

B, N_OBJ, DIM_IN, APK = 16384, 16, 256, 64
N_CORES = 8
BS = B // N_CORES          # 2048 batch rows per core
NT = 512                   # moving-operand tile (one PSUM bank of fp32)
NB = BS // NT              # 4 batch chunks per core
NP = N_OBJ // 2            # 8 object pairs
F32 = mybir.dt.float32
BF16 = mybir.dt.bfloat16
BF16_NP = ml_dtypes.bfloat16

_CACHE: dict = {}

# Pairs with chunk-major layout (loaded/computed in batch-halves): the first
# pair (fast pipeline head) and the last (short drain tail).
EDGE_PAIRS = (0, NP - 1)


def _x_off(op, o2, k, n):
    """Element offset of batch-chunk n of (o2, k) in pair op's flat x line."""
    if op in EDGE_PAIRS:
        # [ch, o2, k, bh] with b = ch*1024 + bh
        return (n // 2) * 4096 + o2 * 2048 + k * 1024 + (n % 2) * NT
    return o2 * 4096 + k * 2048 + n * NT  # [o2, k, b]


def _build_nc(variant=None):
    if variant is None:
        variant = os.environ.get("KVARIANT", "v6")
    n_warm = int(os.environ.get("KWARM", "10"))
    nc = bacc.Bacc("TRN2", target_bir_lowering=False, debug=False)

    xt = nc.declare_dram_parameter("xt", [NP, 128, 8192], BF16, isOutput=False)
    wt = nc.declare_dram_parameter("wt", [128, 2, N_OBJ, APK], BF16, isOutput=False)
    bt = nc.declare_dram_parameter("bt", [128, NP], F32, isOutput=False)
    out = nc.declare_dram_parameter("out", [NP // 2, 128, 2, BS], BF16, isOutput=True)

    with tile.TileContext(nc) as tc, ExitStack() as ctx:
        wpool = ctx.enter_context(tc.tile_pool(name="w", bufs=1))
        xpool = ctx.enter_context(tc.tile_pool(name="x", bufs=NP))
        psum = ctx.enter_context(
            tc.tile_pool(name="ps", bufs=8, space=bass.MemorySpace.PSUM)
        )
        opool = ctx.enter_context(tc.tile_pool(name="o", bufs=2))

        # W/bias via SWDGE (gpsimd) — off the sync queue, may start during
        # SP's register-init window.
        w_sb = wpool.tile([128, 2, N_OBJ, APK], BF16)
        nc.gpsimd.dma_start(w_sb[:], wt[:])
        b_sb = wpool.tile([128, NP], F32)
        nc.gpsimd.dma_start(b_sb[:], bt[:])

        # PE p-state warmup: dep-free matmuls on scratch SBUF ramp the PE
        # clock to full speed while the first x tile streams in.
        if n_warm:
            scratch = wpool.tile([128, APK + NT], BF16)
            nc.vector.memset(scratch[:], 0.0)
            ps_warm = psum.tile([128, NT], F32, name="ps")
            for _ in range(n_warm):
                nc.tensor.matmul(
                    ps_warm[:APK, :],
                    scratch[:, :APK],
                    scratch[:, APK:],
                    start=True,
                    stop=True,
                )

        # All x loads upfront on the sync queue; the DMA engines stream them
        # in issue order, so pair i always lands before pair i+1.
        xts = []
        for op in range(NP):
            t = xpool.tile([128, 8192], BF16)
            if op in EDGE_PAIRS:
                for h in range(2):
                    nc.sync.dma_start(
                        t[:, h * 4096 : (h + 1) * 4096],
                        xt[op, :, h * 4096 : (h + 1) * 4096],
                    )
            else:
                nc.sync.dma_start(t[:], xt[op])
            xts.append(t)

        ot = None
        for op in range(NP):
            xs = xts[op]
            if op % 2 == 0:
                ot = opool.tile([128, 2, BS], BF16)
            pss = [psum.tile([128, NT], F32, name="ps") for n in range(NB)]

            def mm(k, o2, n):
                nc.tensor.matmul(
                    pss[n][o2 * 64 : (o2 + 1) * 64, :],
                    w_sb[:, k, 2 * op + o2, :],
                    xs[:, _x_off(op, o2, k, n) : _x_off(op, o2, k, n) + NT],
                    start=(k == 0),
                    stop=(k == 1),
                )

            def act(n):
                nc.scalar.activation(
                    ot[:, op % 2, n * NT : (n + 1) * NT],
                    pss[n][:],
                    mybir.ActivationFunctionType.Identity,
                    bias=b_sb[:, op : op + 1],
                )

            if op in EDGE_PAIRS:
                # chunk-major halves: compute/evacuate each half as it lands
                for ch in range(2):
                    for k in range(2):
                        for o2 in range(2):
                            for n2 in range(2):
                                mm(k, o2, 2 * ch + n2)
                    for n2 in range(2):
                        act(2 * ch + n2)
                    if op == NP - 1:
                        # store this half immediately (2 KiB lines)
                        nc.scalar.dma_start(
                            out[op // 2, :, op % 2, ch * 1024 : (ch + 1) * 1024],
                            ot[:, op % 2, ch * 1024 : (ch + 1) * 1024],
                        )
            else:
                for k in range(2):
                    for o2 in range(2):
                        for n in range(NB):
                            mm(k, o2, n)
                for n in range(NB):
                    act(n)

            if op % 2 == 1 and op != NP - 1:
                # two pairs per store -> 8 KiB contiguous lines
                nc.scalar.dma_start(out[op // 2], ot[:])
            elif op == NP - 2:
                nc.scalar.dma_start(
                    out[op // 2, :, op % 2, :], ot[:, op % 2, :]
                )

    nc.compile()
    return nc


def _get_nc():
    if "nc" not in _CACHE:
        _CACHE["nc"] = _build_nc()
    return _CACHE["nc"]


def _prep_inputs(x, W, b):
    x = np.ascontiguousarray(x, dtype=np.float32)
    # wt[p, k, o, apk]: W[o,a,par,d,kk] -> [d,o,apk] -> [k,128,o,apk] -> [128,k,o,apk]
    wt = np.ascontiguousarray(
        np.asarray(W, dtype=np.float32)
        .transpose(3, 0, 1, 2, 4)
        .reshape(2, 128, N_OBJ, APK)
        .transpose(1, 0, 2, 3)
        .astype(BF16_NP)
    )
    # bt[o2*64+apk, pair]
    bt = np.ascontiguousarray(
        np.asarray(b, dtype=np.float32)
        .reshape(NP, 2, APK)
        .transpose(1, 2, 0)
        .reshape(128, NP)
    )
    # x[b_all, o, d] with o = pair*2 + o2, d = k*128 + p
    xb = x.reshape(N_CORES, BS, NP, 2, 2, 128).astype(BF16_NP)
    xt_all = np.empty((N_CORES, NP, 128, 8192), dtype=BF16_NP)
    # mid pairs: [core, pair, p, o2, k, b]
    mid = xb.transpose(0, 2, 5, 3, 4, 1)
    # edge pairs: [core, pair, p, ch, o2, k, bh] with b = ch*1024 + bh
    edge = xb.reshape(N_CORES, 2, BS // 2, NP, 2, 2, 128).transpose(
        0, 3, 6, 1, 4, 5, 2
    )
    for op in range(NP):
        src = edge[:, op] if op in EDGE_PAIRS else mid[:, op]
        xt_all[:, op] = src.reshape(N_CORES, 128, 8192)
    in_maps = [
        {"xt": xt_all[c], "wt": wt, "bt": bt} for c in range(N_CORES)
    ]
    return in_maps


def kernel(x, W, b, _trace=False, **run_kwargs):
    nc = _get_nc()
    in_maps = _prep_inputs(x, W, b)
    res = run_bass_kernel_spmd(
        nc, in_maps, core_ids=list(range(N_CORES)), trace=_trace, **run_kwargs
    )
    _CACHE["last_results"] = res
    out = np.empty((B, N_OBJ, APK), dtype=np.float32)
    for c in range(N_CORES):
        # out_t[pp, o2*64+apk, pair2, b] -> [b, (pp,pair2,o2), apk]
        r = np.asarray(res.results[c]["out"]).astype(np.float32)
        r = r.reshape(NP // 2, 2, APK, 2, BS).transpose(4, 0, 3, 1, 2)
        out[c * BS : (c + 1) * BS] = r.reshape(BS, N_OBJ, APK)
    return out.reshape(B, N_OBJ, 4, 2, 8)


# revision 10
# speedup vs baseline: 2.0204x; 1.0052x over previous
"""ObjectDecoder kernel for Trainium2 (8 NeuronCores, data-parallel over batch).

Computes out[b, o, a, p, k] = sum_d x[b, o, d] * W[o, a, p, d, k] + bias[o, a, p, k]
  x: [16384, 16, 256] f32, W: [16, 4, 2, 256, 8] f32, b: [16, 4, 2, 8] f32
  out: [16384, 16, 4, 2, 8] f32

DMA-bound problem: per core the batch shard is 2048 rows -> 33.5 MB of x in
fp32. The 2e-2 rel-err budget is ~5000x above fp32 accuracy, so x/W/out all
move as bf16 (measured end-to-end rel err ~3.4e-3), halving HBM bytes.

Per-core plan (batch shard of 2048 rows):
  - x pre-packed on host to xt[pair, p(128), (o2, k, b)] bf16 with d = k*128+p:
    16 KiB contiguous per partition line, one dma_start per object pair, all
    issued upfront on the sync queue. The 16 DMA engines then stream the full
    16.8 MB back-to-back (~26 GB/s/engine); compute chases the loads.
    Only full-tile dma_starts: partial-tile (sliced) loads put the queue in
    ordered-packet mode, which serializes engines at ~40% duty cycle.
  - The last pair is two separate half-tiles (split on o2) so its matmuls
    and PSUM evacuation overlap the final load instead of draining after it.
  - W + bias load first on the scalar queue (HWDGE; the scalar engine is
    otherwise idle until the first PSUM evacuation at ~8us). gpsimd/SWDGE is
    avoided entirely: its init + software descriptor generation delivered W
    at 18.5us in a previous version.
  - Per pair: 16 matmuls [K=128, M=64, N=512] (bf16, fp32 PSUM accumulate),
    k-outer order; the PE runs the two M=64 column-tiles concurrently, so
    effective matmul cost is ~231 ns — well under the ~5 us/pair DMA.
  - Scalar engine evacuates PSUM with fused per-partition bias add to bf16;
    stores: two pairs per dma_start (8 KiB lines) except the last two pairs,
    stored separately (4 KiB lines) so the final store starts early.
"""

import os
from contextlib import ExitStack

os.environ.setdefault("JAX_PLATFORMS", "axon")

import ml_dtypes
import numpy as np

import concourse.bass as bass
import concourse.mybir as mybir
import concourse.tile as tile
from concourse import bacc
from concourse.bass_utils import run_bass_kernel_spmd

B, N_OBJ, DIM_IN, APK = 16384, 16, 256, 64
N_CORES = 8
BS = B // N_CORES          # 2048 batch rows per core
NT = 512                   # moving-operand tile (one PSUM bank of fp32)
NB = BS // NT              # 4 batch chunks per core
NP = N_OBJ // 2            # 8 object pairs
F32 = mybir.dt.float32
BF16 = mybir.dt.bfloat16
BF16_NP = ml_dtypes.bfloat16

_CACHE: dict = {}


def _build_nc(variant=None):
    if variant is None:
        variant = os.environ.get("KVARIANT", "v7")
    nc = bacc.Bacc("TRN2", target_bir_lowering=False, debug=False)

    xt = nc.declare_dram_parameter("xt", [NP, 128, 8192], BF16, isOutput=False)
    wt = nc.declare_dram_parameter("wt", [128, 2, N_OBJ, APK], BF16, isOutput=False)
    bt = nc.declare_dram_parameter("bt", [128, NP], F32, isOutput=False)
    out = nc.declare_dram_parameter("out", [NP // 2, 128, 2, BS], BF16, isOutput=True)

    with tile.TileContext(nc) as tc, ExitStack() as ctx:
        wpool = ctx.enter_context(tc.tile_pool(name="w", bufs=1))
        xpool = ctx.enter_context(tc.tile_pool(name="x", bufs=NP - 1))
        hpool = ctx.enter_context(tc.tile_pool(name="xh", bufs=2))
        psum = ctx.enter_context(
            tc.tile_pool(name="ps", bufs=8, space=bass.MemorySpace.PSUM)
        )
        opool = ctx.enter_context(tc.tile_pool(name="o", bufs=2))

        # W/bias first on the scalar HWDGE queue (idle until evacuations).
        w_sb = wpool.tile([128, 2, N_OBJ, APK], BF16)
        nc.scalar.dma_start(w_sb[:], wt[:])
        b_sb = wpool.tile([128, NP], F32)
        nc.scalar.dma_start(b_sb[:], bt[:])

        # All x loads upfront on the sync queue; the DMA engines stream them
        # in issue order, so pair i always lands before pair i+1. The last
        # pair is two half-tiles (one per object) so its compute overlaps
        # the tail of the stream.
        xts = []
        for op in range(NP - 1):
            t = xpool.tile([128, 8192], BF16)
            nc.sync.dma_start(t[:], xt[op])
            xts.append(t)
        halves = []
        for h in range(2):
            th = hpool.tile([128, 4096], BF16, name="th")
            nc.sync.dma_start(th[:], xt[NP - 1, :, h * 4096 : (h + 1) * 4096])
            halves.append(th)

        ot = None
        for op in range(NP):
            if op % 2 == 0:
                ot = opool.tile([128, 2, BS], BF16)
            pss = [psum.tile([128, NT], F32, name="ps") for n in range(NB)]

            def mov(o2, k, n):
                # moving operand: [128, NT] slice at (o2, k, n*NT)
                if op == NP - 1:
                    off = k * BS + n * NT
                    return halves[o2][:, off : off + NT]
                off = o2 * 2 * BS + k * BS + n * NT
                return xts[op][:, off : off + NT]

            def mm(k, o2, n):
                nc.tensor.matmul(
                    pss[n][o2 * 64 : (o2 + 1) * 64, :],
                    w_sb[:, k, 2 * op + o2, :],
                    mov(o2, k, n),
                    start=(k == 0),
                    stop=(k == 1),
                )

            def act(n):
                nc.scalar.activation(
                    ot[:, op % 2, n * NT : (n + 1) * NT],
                    pss[n][:],
                    mybir.ActivationFunctionType.Identity,
                    bias=b_sb[:, op : op + 1],
                )

            if op == NP - 1:
                # o2-outer so the o2=0 matmuls run while the o2=1 half lands
                for o2 in range(2):
                    for k in range(2):
                        for n in range(NB):
                            mm(k, o2, n)
            else:
                for k in range(2):
                    for o2 in range(2):
                        for n in range(NB):
                            mm(k, o2, n)
            for n in range(NB):
                act(n)

            if op % 2 == 1 and op < NP - 2:
                # two pairs per store -> 8 KiB contiguous lines
                nc.scalar.dma_start(out[op // 2], ot[:])
            elif op >= NP - 2:
                # last two pairs stored separately (4 KiB lines) so the
                # final store isn't gated on both pairs' evacuations
                nc.scalar.dma_start(
                    out[op // 2, :, op % 2, :], ot[:, op % 2, :]
                )

    nc.compile()
    return nc


def _get_nc():
    if "nc" not in _CACHE:
        _CACHE["nc"] = _build_nc()
    return _CACHE["nc"]


def _prep_inputs(x, W, b):
    x = np.ascontiguousarray(x, dtype=np.float32)
    # wt[p, k, o, apk]: W[o,a,par,d,kk] -> [d,o,apk] -> [k,128,o,apk] -> [128,k,o,apk]
    wt = np.ascontiguousarray(
        np.asarray(W, dtype=np.float32)
        .transpose(3, 0, 1, 2, 4)
        .reshape(2, 128, N_OBJ, APK)
        .transpose(1, 0, 2, 3)
        .astype(BF16_NP)
    )
    # bt[o2*64+apk, pair]
    bt = np.ascontiguousarray(
        np.asarray(b, dtype=np.float32)
        .reshape(NP, 2, APK)
        .transpose(1, 2, 0)
        .reshape(128, NP)
    )
    # x[b_all, o, d] with o = pair*2 + o2, d = k*128 + p
    # -> xt[core, pair, p, o2, k, b]
    xb = x.reshape(N_CORES, BS, NP, 2, 2, 128).astype(BF16_NP)
    xt_all = np.ascontiguousarray(xb.transpose(0, 2, 5, 3, 4, 1)).reshape(
        N_CORES, NP, 128, 8192
    )
    return [{"xt": xt_all[c], "wt": wt, "bt": bt} for c in range(N_CORES)]


def kernel(x, W, b, _trace=False, **run_kwargs):
    nc = _get_nc()
    in_maps = _prep_inputs(x, W, b)
    res = run_bass_kernel_spmd(
        nc, in_maps, core_ids=list(range(N_CORES)), trace=_trace, **run_kwargs
    )
    _CACHE["last_results"] = res
    out = np.empty((B, N_OBJ, APK), dtype=np.float32)
    for c in range(N_CORES):
        # out_t[pp, o2*64+apk, pair2, b] -> [b, (pp,pair2,o2), apk]
        r = np.asarray(res.results[c]["out"]).astype(np.float32)
        r = r.reshape(NP // 2, 2, APK, 2, BS).transpose(4, 0, 3, 1, 2)
        out[c * BS : (c + 1) * BS] = r.reshape(BS, N_OBJ, APK)
    return out.reshape(B, N_OBJ, 4, 2, 8)


# revision 11
# speedup vs baseline: 2.8422x; 1.4067x over previous
"""ObjectDecoder kernel for Trainium2 (8 NeuronCores, data-parallel over batch).

Computes out[b, o, a, p, k] = sum_d x[b, o, d] * W[o, a, p, d, k] + bias[o, a, p, k]
  x: [16384, 16, 256] f32, W: [16, 4, 2, 256, 8] f32, b: [16, 4, 2, 8] f32
  out: [16384, 16, 4, 2, 8] f32

DMA-bound problem: per core the batch shard is 2048 rows -> 33.5 MB of x in
fp32. The 2e-2 rel-err budget is ~5000x above fp32 accuracy, so precision is
traded for HBM bytes: W and out move as bf16, and x moves as fp8 (e3m4,
pre-scaled by 2 with the inverse folded into W) or bf16 depending on X_DT.

Per-core plan (batch shard of 2048 rows):
  - W + bias load FIRST on the sync queue: they stream alone at full rate and
    land by ~7us, so the PE pipeline starts immediately (on a shared queue
    behind x they complete at ~24us and stall everything).
  - x pre-packed on host to xt[pair, p(128), (o2, k, b)] with d = k*128+p:
    one full-tile dma_start per object pair, all issued upfront on the sync
    queue; the 16 DMA engines stream them back-to-back (~26 GB/s/engine) and
    compute chases the loads. The last pair is two half-tiles (split on o2)
    so its matmuls overlap the final load instead of draining after it.
  - Per pair: 16 matmuls [K=128, M=64, N=512] (fp32 PSUM accumulate),
    k-outer order; the PE runs the two M=64 column-tiles concurrently, so
    effective matmul cost is ~231 ns.
  - Scalar engine evacuates PSUM with fused per-partition bias add to bf16;
    stores: two pairs per dma_start (8 KiB lines) except the last two pairs,
    stored separately so the final store starts early. opool bufs=4 keeps
    slow mid-stream stores (they get ~1/3 of the shared DMA bandwidth) from
    back-pressuring the activation pipeline.
"""

import os
from contextlib import ExitStack

os.environ.setdefault("JAX_PLATFORMS", "axon")

import ml_dtypes
import numpy as np

import concourse.bass as bass
import concourse.mybir as mybir
import concourse.tile as tile
from concourse import bacc
from concourse.bass_utils import run_bass_kernel_spmd

B, N_OBJ, DIM_IN, APK = 16384, 16, 256, 64
N_CORES = 8
BS = B // N_CORES          # 2048 batch rows per core
NT = 512                   # moving-operand tile (one PSUM bank of fp32)
NB = BS // NT              # 4 batch chunks per core
NP = N_OBJ // 2            # 8 object pairs
F32 = mybir.dt.float32
BF16 = mybir.dt.bfloat16
BF16_NP = ml_dtypes.bfloat16

# x on-device dtype: "bf16" (rel err ~3.4e-3) or "fp8" (e3m4, rel err ~1.4e-2,
# x scaled by 2 on host with the 1/2 folded into the bf16 W — exact).
X_DT = os.environ.get("KDTYPE", "bf16")
X_MY = {"bf16": BF16, "fp8": mybir.dt.float8e3}[X_DT]
X_NP = {"bf16": BF16_NP, "fp8": ml_dtypes.float8_e3m4}[X_DT]
X_SCALE = {"bf16": 1.0, "fp8": 2.0}[X_DT]

_CACHE: dict = {}


def _build_nc():
    nc = bacc.Bacc("TRN2", target_bir_lowering=False, debug=False)

    xt = nc.declare_dram_parameter("xt", [NP, 128, 8192], X_MY, isOutput=False)
    wt = nc.declare_dram_parameter("wt", [128, 2, N_OBJ, APK], BF16, isOutput=False)
    bt = nc.declare_dram_parameter("bt", [128, NP], F32, isOutput=False)
    out = nc.declare_dram_parameter("out", [NP // 2, 128, 2, BS], BF16, isOutput=True)

    with tile.TileContext(nc) as tc, ExitStack() as ctx:
        wpool = ctx.enter_context(tc.tile_pool(name="w", bufs=1))
        xpool = ctx.enter_context(tc.tile_pool(name="x", bufs=NP - 1))
        hpool = ctx.enter_context(tc.tile_pool(name="xh", bufs=2))
        psum = ctx.enter_context(
            tc.tile_pool(name="ps", bufs=8, space=bass.MemorySpace.PSUM)
        )
        opool = ctx.enter_context(tc.tile_pool(name="o", bufs=4))

        # W/bias stream first, alone, at full rate -> PE starts at ~7us.
        w_sb = wpool.tile([128, 2, N_OBJ, APK], BF16)
        nc.sync.dma_start(w_sb[:], wt[:])
        b_sb = wpool.tile([128, NP], F32)
        nc.sync.dma_start(b_sb[:], bt[:])

        # All x loads upfront on the sync queue; the DMA engines stream them
        # in issue order, so pair i always lands before pair i+1. The last
        # pair is two half-tiles (one per object) so its compute overlaps
        # the tail of the stream.
        xts = []
        for op in range(NP - 1):
            t = xpool.tile([128, 8192], X_MY)
            nc.sync.dma_start(t[:], xt[op])
            xts.append(t)
        halves = []
        for h in range(2):
            th = hpool.tile([128, 4096], X_MY, name="th")
            nc.sync.dma_start(th[:], xt[NP - 1, :, h * 4096 : (h + 1) * 4096])
            halves.append(th)

        ot = None
        for op in range(NP):
            if op % 2 == 0:
                ot = opool.tile([128, 2, BS], BF16)
            pss = [psum.tile([128, NT], F32, name="ps") for n in range(NB)]

            def mov(o2, k, n):
                # moving operand: [128, NT] slice at (o2, k, n*NT)
                if op == NP - 1:
                    off = k * BS + n * NT
                    return halves[o2][:, off : off + NT]
                off = o2 * 2 * BS + k * BS + n * NT
                return xts[op][:, off : off + NT]

            def mm(k, o2, n):
                nc.tensor.matmul(
                    pss[n][o2 * 64 : (o2 + 1) * 64, :],
                    w_sb[:, k, 2 * op + o2, :],
                    mov(o2, k, n),
                    start=(k == 0),
                    stop=(k == 1),
                )

            def act(n):
                nc.scalar.activation(
                    ot[:, op % 2, n * NT : (n + 1) * NT],
                    pss[n][:],
                    mybir.ActivationFunctionType.Identity,
                    bias=b_sb[:, op : op + 1],
                )

            if op == NP - 1:
                # o2-outer so the o2=0 matmuls run while the o2=1 half lands
                for o2 in range(2):
                    for k in range(2):
                        for n in range(NB):
                            mm(k, o2, n)
            else:
                for k in range(2):
                    for o2 in range(2):
                        for n in range(NB):
                            mm(k, o2, n)
            for n in range(NB):
                act(n)

            if op % 2 == 1 and op < NP - 2:
                # two pairs per store -> 8 KiB contiguous lines
                nc.scalar.dma_start(out[op // 2], ot[:])
            elif op >= NP - 2:
                # last two pairs stored separately (4 KiB lines) so the
                # final store isn't gated on both pairs' evacuations
                nc.scalar.dma_start(
                    out[op // 2, :, op % 2, :], ot[:, op % 2, :]
                )

    nc.compile()
    return nc


def _get_nc():
    if "nc" not in _CACHE:
        _CACHE["nc"] = _build_nc()
    return _CACHE["nc"]


def _prep_inputs(x, W, b):
    x = np.ascontiguousarray(x, dtype=np.float32)
    # wt[p, k, o, apk]: W[o,a,par,d,kk] -> [d,o,apk] -> [k,128,o,apk] -> [128,k,o,apk]
    # 1/X_SCALE folded in AFTER bf16 rounding (exact: exponent shift).
    wt = np.ascontiguousarray(
        (
            np.asarray(W, dtype=np.float32)
            .transpose(3, 0, 1, 2, 4)
            .reshape(2, 128, N_OBJ, APK)
            .transpose(1, 0, 2, 3)
            .astype(BF16_NP)
            .astype(np.float32)
            / X_SCALE
        ).astype(BF16_NP)
    )
    # bt[o2*64+apk, pair]
    bt = np.ascontiguousarray(
        np.asarray(b, dtype=np.float32)
        .reshape(NP, 2, APK)
        .transpose(1, 2, 0)
        .reshape(128, NP)
    )
    # x[b_all, o, d] with o = pair*2 + o2, d = k*128 + p
    # -> xt[core, pair, p, o2, k, b]
    xb = (x.reshape(N_CORES, BS, NP, 2, 2, 128) * np.float32(X_SCALE)).astype(X_NP)
    xt_all = np.ascontiguousarray(xb.transpose(0, 2, 5, 3, 4, 1)).reshape(
        N_CORES, NP, 128, 8192
    )
    return [{"xt": xt_all[c], "wt": wt, "bt": bt} for c in range(N_CORES)]


def kernel(x, W, b, _trace=False, **run_kwargs):
    nc = _get_nc()
    in_maps = _prep_inputs(x, W, b)
    res = run_bass_kernel_spmd(
        nc, in_maps, core_ids=list(range(N_CORES)), trace=_trace, **run_kwargs
    )
    _CACHE["last_results"] = res
    out = np.empty((B, N_OBJ, APK), dtype=np.float32)
    for c in range(N_CORES):
        # out_t[pp, o2*64+apk, pair2, b] -> [b, (pp,pair2,o2), apk]
        r = np.asarray(res.results[c]["out"]).astype(np.float32)
        r = r.reshape(NP // 2, 2, APK, 2, BS).transpose(4, 0, 3, 1, 2)
        out[c * BS : (c + 1) * BS] = r.reshape(BS, N_OBJ, APK)
    return out.reshape(B, N_OBJ, 4, 2, 8)


# revision 13
# speedup vs baseline: 3.0799x; 1.0836x over previous
"""ObjectDecoder kernel for Trainium2 (8 NeuronCores, data-parallel over batch).

Computes out[b, o, a, p, k] = sum_d x[b, o, d] * W[o, a, p, d, k] + bias[o, a, p, k]
  x: [16384, 16, 256] f32, W: [16, 4, 2, 256, 8] f32, b: [16, 4, 2, 8] f32
  out: [16384, 16, 4, 2, 8] f32

DMA-bound problem: per core the batch shard is 2048 rows -> 33.5 MB of x in
fp32. The 2e-2 rel-err budget is ~5000x above fp32 accuracy, so precision is
traded for HBM bytes: W and out move as bf16, and x moves as fp8 (e3m4,
pre-scaled by 2 with the inverse folded into W) or bf16 depending on X_DT.

Per-core plan (batch shard of 2048 rows):
  - W + bias load FIRST on the sync queue: they stream alone at full rate and
    land by ~7us, so the PE pipeline starts immediately (on a shared queue
    behind x they complete at ~24us and stall everything).
  - x pre-packed on host to xt[pair, p(128), (o2, k, b)] with d = k*128+p:
    one full-tile dma_start per object pair, all issued upfront on the sync
    queue; the 16 DMA engines stream them back-to-back (~26 GB/s/engine) and
    compute chases the loads. The last pair is two half-tiles (split on o2)
    so its matmuls overlap the final load instead of draining after it.
  - Per pair: 16 matmuls [K=128, M=64, N=512] (fp32 PSUM accumulate),
    k-outer order; the PE runs the two M=64 column-tiles concurrently, so
    effective matmul cost is ~231 ns.
  - Scalar engine evacuates PSUM with fused per-partition bias add to bf16;
    stores: two pairs per dma_start (8 KiB lines) except the last two pairs,
    stored separately so the final store starts early. opool bufs=4 keeps
    slow mid-stream stores (they get ~1/3 of the shared DMA bandwidth) from
    back-pressuring the activation pipeline.
"""

import os
from contextlib import ExitStack

os.environ.setdefault("JAX_PLATFORMS", "axon")

import ml_dtypes
import numpy as np

import concourse.bass as bass
import concourse.mybir as mybir
import concourse.tile as tile
from concourse import bacc
from concourse.bass_utils import run_bass_kernel_spmd

B, N_OBJ, DIM_IN, APK = 16384, 16, 256, 64
N_CORES = 8
BS = B // N_CORES          # 2048 batch rows per core
NT = 512                   # moving-operand tile (one PSUM bank of fp32)
NB = BS // NT              # 4 batch chunks per core
NP = N_OBJ // 2            # 8 object pairs
F32 = mybir.dt.float32
BF16 = mybir.dt.bfloat16
BF16_NP = ml_dtypes.bfloat16

# x on-device dtype: "bf16" (rel err ~3.4e-3) or "fp8" (e3m4, rel err ~1.4e-2,
# x scaled by 2 on host with the 1/2 folded into the bf16 W — exact).
X_DT = os.environ.get("KDTYPE", "bf16")
X_MY = {"bf16": BF16, "fp8": mybir.dt.float8e3}[X_DT]
X_NP = {"bf16": BF16_NP, "fp8": ml_dtypes.float8_e3m4}[X_DT]
X_SCALE = {"bf16": 1.0, "fp8": 2.0}[X_DT]

_CACHE: dict = {}


def _build_nc():
    nc = bacc.Bacc("TRN2", target_bir_lowering=False, debug=False)

    xt = nc.declare_dram_parameter("xt", [NP, 128, 8192], X_MY, isOutput=False)
    wt = nc.declare_dram_parameter("wt", [128, 2, N_OBJ, APK], BF16, isOutput=False)
    bt = nc.declare_dram_parameter("bt", [128, NP], F32, isOutput=False)
    out = nc.declare_dram_parameter("out", [NP // 2, 128, 2, BS], BF16, isOutput=True)

    with tile.TileContext(nc) as tc, ExitStack() as ctx:
        wpool = ctx.enter_context(tc.tile_pool(name="w", bufs=1))
        xpool = ctx.enter_context(tc.tile_pool(name="x", bufs=NP - 1))
        hpool = ctx.enter_context(tc.tile_pool(name="xh", bufs=2))
        psum = ctx.enter_context(
            tc.tile_pool(name="ps", bufs=8, space=bass.MemorySpace.PSUM)
        )
        opool = ctx.enter_context(tc.tile_pool(name="o", bufs=4))

        # Head: interleave W (split by k-chunk) with pair-0 quarter-tiles so
        # the first matmuls (k=0, o2=0) fire as soon as the first ~0.8 MB
        # lands, instead of waiting for all of W plus a full pair.
        wk = [wpool.tile([128, N_OBJ, APK], BF16, name=f"wk{k}") for k in range(2)]
        quarters = {}

        def quarter_load(o2, k):
            q = hpool.tile([128, 2048], X_MY, name="q0", bufs=4)
            nc.sync.dma_start(
                q[:], xt[0, :, (o2 * 2 + k) * 2048 : (o2 * 2 + k + 1) * 2048]
            )
            quarters[o2, k] = q

        nc.sync.dma_start(wk[0][:], wt[:, 0])
        quarter_load(0, 0)
        nc.sync.dma_start(wk[1][:], wt[:, 1])
        quarter_load(0, 1)
        quarter_load(1, 0)
        quarter_load(1, 1)
        b_sb = wpool.tile([128, NP], F32)
        nc.sync.dma_start(b_sb[:], bt[:])

        # Remaining x loads upfront on the sync queue; the DMA engines stream
        # them in issue order, so pair i always lands before pair i+1. The
        # last pair is two half-tiles (one per object) so its compute
        # overlaps the tail of the stream.
        xts = [None]
        for op in range(1, NP - 1):
            t = xpool.tile([128, 8192], X_MY)
            nc.sync.dma_start(t[:], xt[op])
            xts.append(t)
        halves = []
        for h in range(2):
            th = hpool.tile([128, 4096], X_MY, name="th")
            nc.sync.dma_start(th[:], xt[NP - 1, :, h * 4096 : (h + 1) * 4096])
            halves.append(th)

        ot = None
        for op in range(NP):
            if op % 2 == 0:
                ot = opool.tile([128, 2, BS], BF16)
            pss = [psum.tile([128, NT], F32, name="ps") for n in range(NB)]

            def mov(o2, k, n):
                # moving operand: [128, NT] slice at (o2, k, n*NT)
                if op == 0:
                    return quarters[o2, k][:, n * NT : (n + 1) * NT]
                if op == NP - 1:
                    off = k * BS + n * NT
                    return halves[o2][:, off : off + NT]
                off = o2 * 2 * BS + k * BS + n * NT
                return xts[op][:, off : off + NT]

            def mm(k, o2, n):
                nc.tensor.matmul(
                    pss[n][o2 * 64 : (o2 + 1) * 64, :],
                    wk[k][:, 2 * op + o2, :],
                    mov(o2, k, n),
                    start=(k == 0),
                    stop=(k == 1),
                )

            def act(n):
                nc.scalar.activation(
                    ot[:, op % 2, n * NT : (n + 1) * NT],
                    pss[n][:],
                    mybir.ActivationFunctionType.Identity,
                    bias=b_sb[:, op : op + 1],
                )

            if op in (0, NP - 1):
                # o2-outer: matches quarter/half load order so compute
                # starts on the first granule
                for o2 in range(2):
                    for k in range(2):
                        for n in range(NB):
                            mm(k, o2, n)
                            if o2 == 1 and k == 1:
                                act(n)
                                if op == NP - 1:
                                    # per-bank store (1 KiB lines): the
                                    # drain tail is act+store of one bank
                                    nc.scalar.dma_start(
                                        out[op // 2, :, 1, n * NT : (n + 1) * NT],
                                        ot[:, 1, n * NT : (n + 1) * NT],
                                    )
            else:
                for k in range(2):
                    for o2 in range(2):
                        for n in range(NB):
                            mm(k, o2, n)
                            if k == 1 and o2 == 1:
                                act(n)

            if op % 2 == 1 and op < NP - 2:
                # two pairs per store -> 8 KiB contiguous lines
                nc.scalar.dma_start(out[op // 2], ot[:])
            elif op == NP - 2:
                nc.scalar.dma_start(
                    out[op // 2, :, op % 2, :], ot[:, op % 2, :]
                )

    nc.compile()
    return nc


def _get_nc():
    if "nc" not in _CACHE:
        _CACHE["nc"] = _build_nc()
    return _CACHE["nc"]


def _prep_inputs(x, W, b):
    x = np.ascontiguousarray(x, dtype=np.float32)
    # wt[p, k, o, apk]: W[o,a,par,d,kk] -> [d,o,apk] -> [k,128,o,apk] -> [128,k,o,apk]
    # 1/X_SCALE folded in AFTER bf16 rounding (exact: exponent shift).
    wt = np.ascontiguousarray(
        (
            np.asarray(W, dtype=np.float32)
            .transpose(3, 0, 1, 2, 4)
            .reshape(2, 128, N_OBJ, APK)
            .transpose(1, 0, 2, 3)
            .astype(BF16_NP)
            .astype(np.float32)
            / X_SCALE
        ).astype(BF16_NP)
    )
    # bt[o2*64+apk, pair]
    bt = np.ascontiguousarray(
        np.asarray(b, dtype=np.float32)
        .reshape(NP, 2, APK)
        .transpose(1, 2, 0)
        .reshape(128, NP)
    )
    # x[b_all, o, d] with o = pair*2 + o2, d = k*128 + p
    # -> xt[core, pair, p, o2, k, b]
    xb = (x.reshape(N_CORES, BS, NP, 2, 2, 128) * np.float32(X_SCALE)).astype(X_NP)
    xt_all = np.ascontiguousarray(xb.transpose(0, 2, 5, 3, 4, 1)).reshape(
        N_CORES, NP, 128, 8192
    )
    return [{"xt": xt_all[c], "wt": wt, "bt": bt} for c in range(N_CORES)]


def kernel(x, W, b, _trace=False, **run_kwargs):
    nc = _get_nc()
    in_maps = _prep_inputs(x, W, b)
    res = run_bass_kernel_spmd(
        nc, in_maps, core_ids=list(range(N_CORES)), trace=_trace, **run_kwargs
    )
    _CACHE["last_results"] = res
    out = np.empty((B, N_OBJ, APK), dtype=np.float32)
    for c in range(N_CORES):
        # out_t[pp, o2*64+apk, pair2, b] -> [b, (pp,pair2,o2), apk]
        r = np.asarray(res.results[c]["out"]).astype(np.float32)
        r = r.reshape(NP // 2, 2, APK, 2, BS).transpose(4, 0, 3, 1, 2)
        out[c * BS : (c + 1) * BS] = r.reshape(BS, N_OBJ, APK)
    return out.reshape(B, N_OBJ, 4, 2, 8)


# revision 15
# speedup vs baseline: 3.3142x; 1.0761x over previous
"""ObjectDecoder kernel for Trainium2 (8 NeuronCores, data-parallel over batch).

Computes out[b, o, a, p, k] = sum_d x[b, o, d] * W[o, a, p, d, k] + bias[o, a, p, k]
  x: [16384, 16, 256] f32, W: [16, 4, 2, 256, 8] f32, b: [16, 4, 2, 8] f32
  out: [16384, 16, 4, 2, 8] f32

DMA-bound problem: per core the batch shard is 2048 rows -> 33.5 MB of x in
fp32. The 2e-2 rel-err budget is ~5000x above fp32 accuracy, so precision is
traded for HBM bytes: W and out move as bf16, and x moves as fp8 (e3m4,
pre-scaled by 2 with the inverse folded into W) or bf16 depending on X_DT.

Per-core plan (batch shard of 2048 rows):
  - W + bias load FIRST on the sync queue: they stream alone at full rate and
    land by ~7us, so the PE pipeline starts immediately (on a shared queue
    behind x they complete at ~24us and stall everything).
  - x pre-packed on host to xt[pair, p(128), (o2, k, b)] with d = k*128+p:
    one full-tile dma_start per object pair, all issued upfront on the sync
    queue; the 16 DMA engines stream them back-to-back (~26 GB/s/engine) and
    compute chases the loads. The last pair is two half-tiles (split on o2)
    so its matmuls overlap the final load instead of draining after it.
  - Per pair: 16 matmuls [K=128, M=64, N=512] (fp32 PSUM accumulate),
    k-outer order; the PE runs the two M=64 column-tiles concurrently, so
    effective matmul cost is ~231 ns.
  - Scalar engine evacuates PSUM with fused per-partition bias add to bf16;
    stores: two pairs per dma_start (8 KiB lines) except the last two pairs,
    stored separately so the final store starts early. opool bufs=4 keeps
    slow mid-stream stores (they get ~1/3 of the shared DMA bandwidth) from
    back-pressuring the activation pipeline.
"""

import os
from contextlib import ExitStack

os.environ.setdefault("JAX_PLATFORMS", "axon")

import ml_dtypes
import numpy as np

import concourse.bass as bass
import concourse.mybir as mybir
import concourse.tile as tile
from concourse import bacc
from concourse.bass_utils import run_bass_kernel_spmd

B, N_OBJ, DIM_IN, APK = 16384, 16, 256, 64
N_CORES = 8
BS = B // N_CORES          # 2048 batch rows per core
NT = 512                   # moving-operand tile (one PSUM bank of fp32)
NB = BS // NT              # 4 batch chunks per core
NP = N_OBJ // 2            # 8 object pairs
F32 = mybir.dt.float32
BF16 = mybir.dt.bfloat16
BF16_NP = ml_dtypes.bfloat16

# x on-device dtype: "bf16" (rel err ~3.4e-3) or "fp8" (e3m4, rel err ~1.4e-2,
# x scaled by 2 on host with the 1/2 folded into the bf16 W — exact).
X_DT = os.environ.get("KDTYPE", "bf16")
X_MY = {"bf16": BF16, "fp8": mybir.dt.float8e3}[X_DT]
X_NP = {"bf16": BF16_NP, "fp8": ml_dtypes.float8_e3m4}[X_DT]
X_SCALE = {"bf16": 1.0, "fp8": 2.0}[X_DT]

_CACHE: dict = {}


def _build_nc():
    nc = bacc.Bacc("TRN2", target_bir_lowering=False, debug=False)

    xt = nc.declare_dram_parameter("xt", [NP, 128, 8192], X_MY, isOutput=False)
    wt = nc.declare_dram_parameter("wt", [128, 2, N_OBJ, APK], BF16, isOutput=False)
    bt = nc.declare_dram_parameter("bt", [128, NP], F32, isOutput=False)
    out = nc.declare_dram_parameter("out", [NP // 2, 128, 2, BS], BF16, isOutput=True)

    with tile.TileContext(nc) as tc, ExitStack() as ctx:
        wpool = ctx.enter_context(tc.tile_pool(name="w", bufs=1))
        xpool = ctx.enter_context(tc.tile_pool(name="x", bufs=NP - 1))
        hpool = ctx.enter_context(tc.tile_pool(name="xh", bufs=2))
        psum = ctx.enter_context(
            tc.tile_pool(name="ps", bufs=8, space=bass.MemorySpace.PSUM)
        )
        opool = ctx.enter_context(tc.tile_pool(name="o", bufs=4))

        # Head: interleave W (split by k-chunk) with pair-0 quarter-tiles so
        # the first matmuls (k=0, o2=0) fire as soon as the first ~0.8 MB
        # lands, instead of waiting for all of W plus a full pair.
        wk = [wpool.tile([128, N_OBJ, APK], BF16, name=f"wk{k}") for k in range(2)]
        quarters = {}

        def quarter_load(o2, k):
            q = hpool.tile([128, 2048], X_MY, name="q0", bufs=4)
            nc.sync.dma_start(
                q[:], xt[0, :, (o2 * 2 + k) * 2048 : (o2 * 2 + k + 1) * 2048]
            )
            quarters[o2, k] = q

        nc.sync.dma_start(wk[0][:], wt[:, 0])
        quarter_load(0, 0)
        nc.sync.dma_start(wk[1][:], wt[:, 1])
        quarter_load(0, 1)
        quarter_load(1, 0)
        quarter_load(1, 1)
        b_sb = wpool.tile([128, NP], F32)
        nc.sync.dma_start(b_sb[:], bt[:])

        # Remaining x loads upfront on the sync queue; the DMA engines stream
        # them in issue order, so pair i always lands before pair i+1. The
        # last pair is two half-tiles (one per object) so its compute
        # overlaps the tail of the stream.
        xts = [None]
        for op in range(1, NP - 1):
            t = xpool.tile([128, 8192], X_MY)
            nc.sync.dma_start(t[:], xt[op])
            xts.append(t)
        halves = []
        for h in range(2):
            th = hpool.tile([128, 4096], X_MY, name="th")
            nc.sync.dma_start(th[:], xt[NP - 1, :, h * 4096 : (h + 1) * 4096])
            halves.append(th)

        ot = None
        for op in range(NP):
            if op % 2 == 0:
                ot = opool.tile([128, 2, BS], BF16)
            pss = [psum.tile([128, NT], F32, name="ps") for n in range(NB)]

            def mov(o2, k, n):
                # moving operand: [128, NT] slice at (o2, k, n*NT)
                if op == 0:
                    return quarters[o2, k][:, n * NT : (n + 1) * NT]
                if op == NP - 1:
                    off = k * BS + n * NT
                    return halves[o2][:, off : off + NT]
                off = o2 * 2 * BS + k * BS + n * NT
                return xts[op][:, off : off + NT]

            def mm(k, o2, n):
                nc.tensor.matmul(
                    pss[n][o2 * 64 : (o2 + 1) * 64, :],
                    wk[k][:, 2 * op + o2, :],
                    mov(o2, k, n),
                    start=(k == 0),
                    stop=(k == 1),
                )

            def act(n):
                # PSUM evacuation split across scalar and vector engines —
                # 32 evacuations on scalar alone (~27us) would out-pace the
                # PE (~29us) and serialize the drain tail.
                dst = ot[:, op % 2, n * NT : (n + 1) * NT]
                if n % 2 == 0:
                    nc.scalar.activation(
                        dst,
                        pss[n][:],
                        mybir.ActivationFunctionType.Identity,
                        bias=b_sb[:, op : op + 1],
                    )
                else:
                    nc.vector.tensor_scalar_add(dst, pss[n][:], b_sb[:, op : op + 1])

            if op in (0, NP - 1):
                # o2-outer: matches quarter/half load order so compute
                # starts on the first granule
                for o2 in range(2):
                    for k in range(2):
                        for n in range(NB):
                            mm(k, o2, n)
                            if o2 == 1 and k == 1:
                                act(n)
                                if op == NP - 1:
                                    # per-bank store (1 KiB lines): the
                                    # drain tail is act+store of one bank
                                    nc.sync.dma_start(
                                        out[op // 2, :, 1, n * NT : (n + 1) * NT],
                                        ot[:, 1, n * NT : (n + 1) * NT],
                                    )
            else:
                for k in range(2):
                    for o2 in range(2):
                        for n in range(NB):
                            mm(k, o2, n)
                            if k == 1 and o2 == 1:
                                act(n)

            if op % 2 == 1 and op < NP - 2:
                # two pairs per store -> 8 KiB contiguous lines
                nc.sync.dma_start(out[op // 2], ot[:])
            elif op == NP - 2:
                nc.sync.dma_start(
                    out[op // 2, :, op % 2, :], ot[:, op % 2, :]
                )

    nc.compile()
    return nc


def _get_nc():
    if "nc" not in _CACHE:
        _CACHE["nc"] = _build_nc()
    return _CACHE["nc"]


def _prep_inputs(x, W, b):
    x = np.ascontiguousarray(x, dtype=np.float32)
    # wt[p, k, o, apk]: W[o,a,par,d,kk] -> [d,o,apk] -> [k,128,o,apk] -> [128,k,o,apk]
    # 1/X_SCALE folded in AFTER bf16 rounding (exact: exponent shift).
    wt = np.ascontiguousarray(
        (
            np.asarray(W, dtype=np.float32)
            .transpose(3, 0, 1, 2, 4)
            .reshape(2, 128, N_OBJ, APK)
            .transpose(1, 0, 2, 3)
            .astype(BF16_NP)
            .astype(np.float32)
            / X_SCALE
        ).astype(BF16_NP)
    )
    # bt[o2*64+apk, pair]
    bt = np.ascontiguousarray(
        np.asarray(b, dtype=np.float32)
        .reshape(NP, 2, APK)
        .transpose(1, 2, 0)
        .reshape(128, NP)
    )
    # x[b_all, o, d] with o = pair*2 + o2, d = k*128 + p
    # -> xt[core, pair, p, o2, k, b]
    xb = (x.reshape(N_CORES, BS, NP, 2, 2, 128) * np.float32(X_SCALE)).astype(X_NP)
    xt_all = np.ascontiguousarray(xb.transpose(0, 2, 5, 3, 4, 1)).reshape(
        N_CORES, NP, 128, 8192
    )
    return [{"xt": xt_all[c], "wt": wt, "bt": bt} for c in range(N_CORES)]


def kernel(x, W, b, _trace=False, **run_kwargs):
    nc = _get_nc()
    in_maps = _prep_inputs(x, W, b)
    res = run_bass_kernel_spmd(
        nc, in_maps, core_ids=list(range(N_CORES)), trace=_trace, **run_kwargs
    )
    _CACHE["last_results"] = res
    out = np.empty((B, N_OBJ, APK), dtype=np.float32)
    for c in range(N_CORES):
        # out_t[pp, o2*64+apk, pair2, b] -> [b, (pp,pair2,o2), apk]
        r = np.asarray(res.results[c]["out"]).astype(np.float32)
        r = r.reshape(NP // 2, 2, APK, 2, BS).transpose(4, 0, 3, 1, 2)
        out[c * BS : (c + 1) * BS] = r.reshape(BS, N_OBJ, APK)
    return out.reshape(B, N_OBJ, 4, 2, 8)


# revision 16
# speedup vs baseline: 3.3455x; 1.0094x over previous
"""ObjectDecoder kernel for Trainium2 (8 NeuronCores, data-parallel over batch).

Computes out[b, o, a, p, k] = sum_d x[b, o, d] * W[o, a, p, d, k] + bias[o, a, p, k]
  x: [16384, 16, 256] f32, W: [16, 4, 2, 256, 8] f32, b: [16, 4, 2, 8] f32
  out: [16384, 16, 4, 2, 8] f32

DMA-bound problem: per core the batch shard is 2048 rows -> 33.5 MB of x in
fp32. The 2e-2 rel-err budget is ~5000x above fp32 accuracy, so precision is
traded for HBM bytes: W and out move as bf16, and x moves as fp8 (e3m4,
pre-scaled by 2 with the inverse folded into W) or bf16 depending on X_DT.

Per-core plan (batch shard of 2048 rows):
  - W + bias load FIRST on the sync queue: they stream alone at full rate and
    land by ~7us, so the PE pipeline starts immediately (on a shared queue
    behind x they complete at ~24us and stall everything).
  - x pre-packed on host to xt[pair, p(128), (o2, k, b)] with d = k*128+p:
    one full-tile dma_start per object pair, all issued upfront on the sync
    queue; the 16 DMA engines stream them back-to-back (~26 GB/s/engine) and
    compute chases the loads. The last pair is two half-tiles (split on o2)
    so its matmuls overlap the final load instead of draining after it.
  - Per pair: 16 matmuls [K=128, M=64, N=512] (fp32 PSUM accumulate),
    k-outer order; the PE runs the two M=64 column-tiles concurrently, so
    effective matmul cost is ~231 ns.
  - Scalar engine evacuates PSUM with fused per-partition bias add to bf16;
    stores: two pairs per dma_start (8 KiB lines) except the last two pairs,
    stored separately so the final store starts early. opool bufs=4 keeps
    slow mid-stream stores (they get ~1/3 of the shared DMA bandwidth) from
    back-pressuring the activation pipeline.
"""

import os
from contextlib import ExitStack

os.environ.setdefault("JAX_PLATFORMS", "axon")

import ml_dtypes
import numpy as np

import concourse.bass as bass
import concourse.mybir as mybir
import concourse.tile as tile
from concourse import bacc
from concourse.bass_utils import run_bass_kernel_spmd

B, N_OBJ, DIM_IN, APK = 16384, 16, 256, 64
N_CORES = 8
BS = B // N_CORES          # 2048 batch rows per core
NT = 512                   # moving-operand tile (one PSUM bank of fp32)
NB = BS // NT              # 4 batch chunks per core
NP = N_OBJ // 2            # 8 object pairs
F32 = mybir.dt.float32
BF16 = mybir.dt.bfloat16
BF16_NP = ml_dtypes.bfloat16

# x on-device dtype: "bf16" (rel err ~3.4e-3) or "fp8" (e3m4, rel err ~1.4e-2,
# x scaled by 2 on host with the 1/2 folded into the bf16 W — exact).
X_DT = os.environ.get("KDTYPE", "bf16")
X_MY = {"bf16": BF16, "fp8": mybir.dt.float8e3}[X_DT]
X_NP = {"bf16": BF16_NP, "fp8": ml_dtypes.float8_e3m4}[X_DT]
X_SCALE = {"bf16": 1.0, "fp8": 2.0}[X_DT]

_CACHE: dict = {}


def _build_nc():
    nc = bacc.Bacc("TRN2", target_bir_lowering=False, debug=False)

    xt = nc.declare_dram_parameter("xt", [NP, 128, 8192], X_MY, isOutput=False)
    wt = nc.declare_dram_parameter("wt", [128, 2, N_OBJ, APK], BF16, isOutput=False)
    bt = nc.declare_dram_parameter("bt", [128, NP], F32, isOutput=False)
    out = nc.declare_dram_parameter("out", [NP // 2, 128, 2, BS], BF16, isOutput=True)

    with tile.TileContext(nc) as tc, ExitStack() as ctx:
        wpool = ctx.enter_context(tc.tile_pool(name="w", bufs=1))
        xpool = ctx.enter_context(tc.tile_pool(name="x", bufs=NP - 1))
        hpool = ctx.enter_context(tc.tile_pool(name="xh", bufs=2))
        psum = ctx.enter_context(
            tc.tile_pool(name="ps", bufs=8, space=bass.MemorySpace.PSUM)
        )
        opool = ctx.enter_context(tc.tile_pool(name="o", bufs=4))

        # Head: interleave W (split by k-chunk) with pair-0 quarter-tiles so
        # the first matmuls (k=0, o2=0) fire as soon as the first ~0.8 MB
        # lands, instead of waiting for all of W plus a full pair.
        wk = [wpool.tile([128, N_OBJ, APK], BF16, name=f"wk{k}") for k in range(2)]
        quarters = {}

        def quarter_load(o2, k):
            q = hpool.tile([128, 2048], X_MY, name="q0", bufs=4)
            nc.sync.dma_start(
                q[:], xt[0, :, (o2 * 2 + k) * 2048 : (o2 * 2 + k + 1) * 2048]
            )
            quarters[o2, k] = q

        nc.sync.dma_start(wk[0][:], wt[:, 0])
        quarter_load(0, 0)
        nc.sync.dma_start(wk[1][:], wt[:, 1])
        quarter_load(0, 1)
        quarter_load(1, 0)
        quarter_load(1, 1)
        b_sb = wpool.tile([128, NP], F32)
        nc.sync.dma_start(b_sb[:], bt[:])

        # Remaining x loads upfront on the sync queue; the DMA engines stream
        # them in issue order, so pair i always lands before pair i+1. The
        # last pair is two half-tiles (one per object) so its compute
        # overlaps the tail of the stream.
        xts = [None]
        for op in range(1, NP - 1):
            t = xpool.tile([128, 8192], X_MY)
            nc.sync.dma_start(t[:], xt[op])
            xts.append(t)
        halves = []
        for h in range(2):
            th = hpool.tile([128, 4096], X_MY, name="th")
            nc.sync.dma_start(th[:], xt[NP - 1, :, h * 4096 : (h + 1) * 4096])
            halves.append(th)

        ot = None
        for op in range(NP):
            if op % 2 == 0:
                ot = opool.tile([128, 2, BS], BF16)
            pss = [psum.tile([128, NT], F32, name="ps") for n in range(NB)]

            def mov(o2, k, n):
                # moving operand: [128, NT] slice at (o2, k, n*NT)
                if op == 0:
                    return quarters[o2, k][:, n * NT : (n + 1) * NT]
                if op == NP - 1:
                    off = k * BS + n * NT
                    return halves[o2][:, off : off + NT]
                off = o2 * 2 * BS + k * BS + n * NT
                return xts[op][:, off : off + NT]

            def mm(k, o2, n):
                nc.tensor.matmul(
                    pss[n][o2 * 64 : (o2 + 1) * 64, :],
                    wk[k][:, 2 * op + o2, :],
                    mov(o2, k, n),
                    start=(k == 0),
                    stop=(k == 1),
                )

            def act(n):
                # PSUM evacuation split across scalar and vector engines —
                # 32 evacuations on scalar alone (~27us) would out-pace the
                # PE (~29us) and serialize the drain tail.
                dst = ot[:, op % 2, n * NT : (n + 1) * NT]
                if n % 2 == 0:
                    nc.scalar.activation(
                        dst,
                        pss[n][:],
                        mybir.ActivationFunctionType.Identity,
                        bias=b_sb[:, op : op + 1],
                    )
                else:
                    nc.vector.tensor_scalar_add(dst, pss[n][:], b_sb[:, op : op + 1])

            if op in (0, NP - 1):
                # o2-outer: matches quarter/half load order so compute
                # starts on the first granule
                for o2 in range(2):
                    for k in range(2):
                        for n in range(NB):
                            mm(k, o2, n)
                            if o2 == 1 and k == 1:
                                act(n)
                                if op == NP - 1 and n % 2 == 1:
                                    # store per batch-half (2 KiB lines) so
                                    # the drain tail is one act + 0.26 MB
                                    hs = (n - 1) * NT
                                    nc.sync.dma_start(
                                        out[op // 2, :, 1, hs : hs + 2 * NT],
                                        ot[:, 1, hs : hs + 2 * NT],
                                    )
            else:
                for k in range(2):
                    for o2 in range(2):
                        for n in range(NB):
                            mm(k, o2, n)
                            if k == 1 and o2 == 1:
                                act(n)

            if op % 2 == 1 and op < NP - 2:
                # two pairs per store -> 8 KiB contiguous lines
                nc.sync.dma_start(out[op // 2], ot[:])
            elif op == NP - 2:
                nc.sync.dma_start(
                    out[op // 2, :, op % 2, :], ot[:, op % 2, :]
                )

    nc.compile()
    return nc


def _get_nc():
    if "nc" not in _CACHE:
        _CACHE["nc"] = _build_nc()
    return _CACHE["nc"]


def _prep_inputs(x, W, b):
    x = np.ascontiguousarray(x, dtype=np.float32)
    # wt[p, k, o, apk]: W[o,a,par,d,kk] -> [d,o,apk] -> [k,128,o,apk] -> [128,k,o,apk]
    # 1/X_SCALE folded in AFTER bf16 rounding (exact: exponent shift).
    wt = np.ascontiguousarray(
        (
            np.asarray(W, dtype=np.float32)
            .transpose(3, 0, 1, 2, 4)
            .reshape(2, 128, N_OBJ, APK)
            .transpose(1, 0, 2, 3)
            .astype(BF16_NP)
            .astype(np.float32)
            / X_SCALE
        ).astype(BF16_NP)
    )
    # bt[o2*64+apk, pair]
    bt = np.ascontiguousarray(
        np.asarray(b, dtype=np.float32)
        .reshape(NP, 2, APK)
        .transpose(1, 2, 0)
        .reshape(128, NP)
    )
    # x[b_all, o, d] with o = pair*2 + o2, d = k*128 + p
    # -> xt[core, pair, p, o2, k, b]
    xb = (x.reshape(N_CORES, BS, NP, 2, 2, 128) * np.float32(X_SCALE)).astype(X_NP)
    xt_all = np.ascontiguousarray(xb.transpose(0, 2, 5, 3, 4, 1)).reshape(
        N_CORES, NP, 128, 8192
    )
    return [{"xt": xt_all[c], "wt": wt, "bt": bt} for c in range(N_CORES)]


def kernel(x, W, b, _trace=False, **run_kwargs):
    nc = _get_nc()
    in_maps = _prep_inputs(x, W, b)
    res = run_bass_kernel_spmd(
        nc, in_maps, core_ids=list(range(N_CORES)), trace=_trace, **run_kwargs
    )
    _CACHE["last_results"] = res
    out = np.empty((B, N_OBJ, APK), dtype=np.float32)
    for c in range(N_CORES):
        # out_t[pp, o2*64+apk, pair2, b] -> [b, (pp,pair2,o2), apk]
        r = np.asarray(res.results[c]["out"]).astype(np.float32)
        r = r.reshape(NP // 2, 2, APK, 2, BS).transpose(4, 0, 3, 1, 2)
        out[c * BS : (c + 1) * BS] = r.reshape(BS, N_OBJ, APK)
    return out.reshape(B, N_OBJ, 4, 2, 8)
